# revision 82
# baseline (speedup 1.0000x reference)
"""KernelCRPS loss on 8 Trainium2 NeuronCores (Bass/Tile).

Math: for each grid point with ensemble p_0..p_15 and target t,
  kcrps = [ mean_k |t - p_k|  - 1/(2*E^2) * sum_{i,j} |p_i - p_j| ] * scale_v * w_p
summed over all points, divided by (sum(w) * batch).

The host prescales yh = fp16(g*y), th = fp16(g*t) with g = scale_v * w_p >= 0.
Per grid point the device needs 120 pair values |yh_i - yh_j| (i<j) and 16
mae values |th - yh_k|.  Points (columns) are split across three paths sized
so ACT / DVE / GPSIMD all finish together:

  PE  cols: a fixed {0,+-1} (17, 128) weight matrix turns each moving column
      (16 ensemble values + th) into 112 pair diffs + 16 mae diffs in PSUM;
      ScalarE Abs+accum reduces 4-bank groups (exact).  The 8 pair rows that
      did not fit (PSUM has 128 partitions) are evaluated over the PE range
      with DVE TT max + ts sum-accum and an exact host-side linear
      correction (|a-b| = 2 max(a,b) - (a+b)).
  GPS cols: GPSIMD computes all 120 pair diffs (15 per-offset TT subtracts,
      1.98 ns/col); DVE consumes with single-pass ts relu+accum @4x
      (|d| = 2 relu(d) - d, linear part corrected host-side).
  SORT cols: DVE sorts the 16 ensemble values with a Batcher odd-even
      network (63 comparators in 10 layers; each layer is one strided
      multi-plane TT min + TT max @2x, ping-ponging between two 16-plane
      regions), then sum_{i<j}|p_i - p_j| = sum_k (2k-15) p_(k) via 16
      ts mult+accum @4x.  No host correction needed for the pair term.

  mae for GPS/SORT cols: DVE TT max(th, y_k) + ts sum-accum with the exact
  host-side correction sum|t-y| = 2 sum max(t,y) - (16 T1 + sum C).

Sharding: latlon 40320 -> 8 cores x 5040 (pointwise per grid point; host
sums per-core partials).
"""

import os

import numpy as np

B, V, P, E = 2, 16, 40320, 16
NCORES = 8
PC = P // NCORES            # 5040 latlon points per core
NPT = B * V * PC            # 161280 (b, v, p) points per core
PART = 128
FREE = NPT // PART          # 1260 points per partition
PE_GROUP = int(os.environ.get("KCRPS_PE_GROUP", "2048"))  # PSUM cols per ACT

_CACHE = {}
LAST_EXEC_NS = None
LAST_NC = None


def _pe_w():
    w = int(os.environ.get("KCRPS_PE_W", "512"))
    assert w % 16 == 0 and 0 <= w < FREE
    return w


def _gps_ws():
    v = os.environ.get("KCRPS_GPS_W", "124,124")
    return [int(x) for x in v.split(",") if x.strip()]


def _sort_ws():
    v = os.environ.get("KCRPS_SORT_W", "")
    if v:
        return [int(x) for x in v.split(",") if x.strip()]
    rest = FREE - _pe_w() - sum(_gps_ws())
    assert rest > 0
    return [rest]


# The 8 pair rows dropped from the 128-row PE matrix (PSUM has 128
# partitions; 120 pairs + 16 mae = 136 > 128).  Their planes for the PE
# point range arrive as a packed 7-plane chunk.
PE_DROP = [(12, 0), (12, 1), (13, 0), (13, 1), (13, 2),
           (14, 0), (14, 1), (15, 0)]
PE_DROP_PLANES = [0, 1, 2, 12, 13, 14, 15]
PE_PAIRS = [(d, i) for d in range(1, E) for i in range(E - d)
            if (d, i) not in PE_DROP]
assert len(PE_PAIRS) == 112

# Batcher odd-even mergesort network for 16 inputs: 63 comparators in 10
# layers.  Each layer: (grid, i_slice, j_slice, passthrough_slices) where
# grid reshapes the 16-plane axis; slices index (outer, inner) plane dims.
# A comparator set {(i, i+d)} maps to one TT min (out=i-planes) + one TT
# max (out=j-planes); untouched planes are copied to the destination
# region with ts bypass @4x.
#   grid "16"  -> planes axis stays 1-D [16]
#   grid "2x8" -> planes viewed [2, 8] (outer stride 8)
#   grid "4x4" -> planes viewed [4, 4] (outer stride 4)
#   grid "8x2" -> planes viewed [8, 2] (outer stride 2)
S16 = (slice(None),)
# raw comparator layers (d, i-list); layer 0 handled specially (reads the
# DMA-in tile in two plane halves)
BATCHER_PAIRS = [
    (1, [0, 2, 4, 6, 8, 10, 12, 14]),
    (2, [0, 1, 4, 5, 8, 9, 12, 13]),
    (1, [1, 5, 9, 13]),
    (4, [0, 1, 2, 3, 8, 9, 10, 11]),
    (2, [2, 3, 10, 11]),
    (1, [1, 3, 5, 9, 11, 13]),
    (8, [0, 1, 2, 3, 4, 5, 6, 7]),
    (4, [4, 5, 6, 7]),
    (2, [2, 3, 6, 7, 10, 11]),
    (1, [1, 3, 5, 7, 9, 11, 13]),
]

_GRIDS = {"16": (16, 1), "2x8": (2, 8), "4x4": (4, 4), "8x2": (8, 2)}


def _express(planes):
    """Find (grid, outer_slice, inner_slice) whose row-major traversal
    yields exactly `planes` (an increasing tuple)."""
    planes = tuple(planes)
    n = len(planes)
    for gname, (ga, gb) in _GRIDS.items():
        for ocnt in range(1, ga + 1):
            if n % ocnt:
                continue
            icnt = n // ocnt
            if icnt > gb:
                continue
            for o0 in range(ga):
                osteps = range(1, ga) if ocnt > 1 else (1,)
                for ostep in osteps:
                    if o0 + (ocnt - 1) * ostep >= ga:
                        continue
                    for i0 in range(gb):
                        isteps = range(1, gb) if icnt > 1 else (1,)
                        for istep in isteps:
                            if i0 + (icnt - 1) * istep >= gb:
                                continue
                            s = tuple(o * gb + i0 + k * istep
                                      for o in range(o0, o0 + ocnt * ostep,
                                                     ostep)
                                      for k in range(icnt))
                            if s == planes:
                                return (
                                    gname,
                                    slice(o0, o0 + (ocnt - 1) * ostep + 1,
                                          ostep),
                                    slice(i0, i0 + (icnt - 1) * istep + 1,
                                          istep))
    return None


def _express_or_split(planes):
    """Express `planes` as >=1 (grid, osl, isl) groups."""
    e = _express(planes)
    if e is not None:
        return [(e, tuple(planes))]
    assert len(planes) > 1, f"cannot express {planes}"
    h = len(planes) // 2
    return (_express_or_split(planes[:h])
            + _express_or_split(planes[h:]))


def _plan_scattered():
    """Plan the Batcher network with per-plane buffer tracking (A=0, B=1)
    and no passthrough copies: compared planes always write to the
    opposite buffer, untouched planes stay put.  Layer 0 reads the DMA
    tile and writes everything to A.

    Returns (layer_ops, b_final): layer_ops = list (per layer 1..9) of
    sub-ops (bi, bj, expr_i, expr_j) where expr = (grid, osl, isl) view
    slices for the i-planes / j-planes; b_final[p] = buffer of sorted
    plane p."""
    half = os.environ.get("KCRPS_SORT_INPLACE", "0") == "1"
    b = [0] * E          # after layer 0 everything is in A
    layer_ops = []
    for d, ilist in BATCHER_PAIRS[1:]:
        groups = {}
        for i in ilist:
            groups.setdefault((b[i], b[i + d]), []).append(i)
        ops = []
        for (bi, bj), iset in sorted(groups.items()):
            for expr_i, pl in _express_or_split(tuple(sorted(iset))):
                jpl = tuple(p + d for p in pl)
                sub = _express_or_split(jpl)
                if len(sub) == 1:
                    ops.append((bi, bj, expr_i, sub[0][0], pl, jpl))
                else:
                    # split i to match j's split granularity
                    for expr_j, jp in sub:
                        ip = tuple(p - d for p in jp)
                        ei = _express(ip)
                        assert ei is not None
                        ops.append((bi, bj, ei, expr_j, ip, jp))
        layer_ops.append(ops)
        for i in ilist:
            # half-in-place: min writes the opposite buffer, max writes
            # in place (out == in1; the min op reads the originals first
            # and writes elsewhere, so ordering on the one engine is safe)
            b[i] ^= 1
            if not half:
                b[i + d] ^= 1
    return layer_ops, b


def _check_scattered():
    """Zero-one-principle check of the scattered plan (all 2^16 inputs,
    vectorized)."""
    half = os.environ.get("KCRPS_SORT_INPLACE", "0") == "1"
    layer_ops, b_final = _plan_scattered()
    nvec = 1 << E
    vals = ((np.arange(nvec, dtype=np.uint32)[:, None]
             >> np.arange(E)[None, :]) & 1).astype(np.int8)
    A = vals.copy()          # layer 0: sorted pairs written to A
    B = np.zeros_like(A)
    for i in range(0, E, 2):
        lo = np.minimum(vals[:, i], vals[:, i + 1])
        hi = np.maximum(vals[:, i], vals[:, i + 1])
        A[:, i], A[:, i + 1] = lo, hi
    bufs = [A, B]
    for ops in layer_ops:
        writes = []
        for bi, bj, _, _, pl, jpl in ops:
            vi = bufs[bi][:, list(pl)]
            vj = bufs[bj][:, list(jpl)]
            writes.append((1 - bi, pl, np.minimum(vi, vj)))
            writes.append((bj if half else 1 - bj, jpl,
                           np.maximum(vi, vj)))
        for wb, wpl, wv in writes:
            bufs[wb][:, list(wpl)] = wv
    out = np.stack([bufs[b_final[p]][:, p] for p in range(E)], axis=1)
    assert (np.diff(out, axis=1) >= 0).all(), "scattered plan does not sort"


_check_scattered()
_SCATTER_OPS, _B_FINAL = _plan_scattered()


def _build_nc(pe_w, gps_ws, sort_ws):
    import concourse.bacc as bacc
    from concourse import mybir, tile
    from concourse.mybir import AluOpType

    f16 = mybir.dt.float16
    f32 = mybir.dt.float32

    n_drop = len(PE_DROP_PLANES)
    sbuf_cols = sum(gps_ws) + sum(sort_ws)
    assert pe_w + sbuf_cols == FREE
    y_cols = sbuf_cols * E + (n_drop * pe_w if pe_w else 0)
    n_pe_groups = PART * pe_w // PE_GROUP if pe_w else 0
    relu_g = int(os.environ.get("KCRPS_RELU_GROUPS", "3"))
    # pb plane groups per gps chunk, split at d boundaries.  Front-loaded
    # (last group small) so the final DVE consume right after GPSIMD
    # finishes is short.
    d_sizes = [E - d for d in range(1, E)]
    d_off = list(np.concatenate([[0], np.cumsum(d_sizes)]))
    d_cuts = os.environ.get("KCRPS_RELU_CUTS", "4,9")
    cuts = [int(x) for x in d_cuts.split(",") if x.strip()]
    assert len(cuts) == relu_g - 1
    bounds = [0] + [int(d_off[c]) for c in cuts] + [120]
    pb_groups = [(bounds[i], bounds[i + 1]) for i in range(relu_g)]

    # per-chunk accumulator column kinds (emission order)
    kinds_gps = ["gpsrelu"] * relu_g + ["gmae"]
    kinds_sort = ["coef%d" % k for k in range(E)] + ["smae"]
    # group 0's activation is split so ACT starts after only half the
    # matmuls; it uses one extra accumulator column
    g0_split = int(os.environ.get("KCRPS_G0_SPLIT", "1"))
    n_pe_cols = (n_pe_groups - 1 + g0_split) if pe_w else 0
    ncol = (len(kinds_gps) * len(gps_ws) + len(kinds_sort) * len(sort_ws)
            + n_pe_cols + (1 if pe_w else 0))

    nc = bacc.Bacc(
        "TRN2",
        target_bir_lowering=False,
        debug=False,
        enable_asserts=False,
        num_devices=NCORES,
    )
    y = nc.dram_tensor("y", [PART, y_cols], f16, kind="ExternalInput")
    t = nc.dram_tensor("t", [PART, FREE], f16, kind="ExternalInput")
    if pe_w:
        wd = nc.dram_tensor("wm", [E + 1, PART], f16, kind="ExternalInput")
        mv = nc.dram_tensor("mv", [E + 1, PART * pe_w], f16,
                            kind="ExternalInput")
    out = nc.dram_tensor("acc", [PART, ncol], f32, kind="ExternalOutput")

    mv_blk = int(os.environ.get("KCRPS_MV_BLK", "2"))  # PE groups per mv DMA

    with tile.TileContext(nc) as tc:
        with (
            tc.tile_pool(name="y_pool", bufs=2) as y_pool,
            tc.tile_pool(name="ys_pool",
                         bufs=min(2, len(sort_ws))) as ys_pool,
            tc.tile_pool(name="pb_pool", bufs=len(gps_ws)) as pb_pool,
            tc.tile_pool(name="st_pool",
                         bufs=min(2, len(sort_ws))) as st_pool,
            tc.tile_pool(name="sc_pool", bufs=2) as sc_pool,
            tc.tile_pool(name="sm_pool",
                         bufs=min(2, len(sort_ws))) as sm_pool,
            tc.tile_pool(name="dr_pool", bufs=1) as dr_pool,
            tc.tile_pool(name="mv_pool", bufs=3) as mv_pool,
            tc.tile_pool(name="mv0_pool", bufs=1) as mv0_pool,
            tc.psum_pool(name="ps_pool", bufs=8192 // PE_GROUP // 2) as ps_pool,
            tc.tile_pool(name="fix", bufs=1) as fix,
        ):
            th = fix.tile([PART, FREE], f16)
            acc = fix.tile([PART, ncol], f32)
            nc.vector.memset(acc[:], 0.0)
            wt = None
            if pe_w:
                wt = fix.tile([E + 1, PART], f16)

            col = [0]
            pe_state = {"next": 0,
                        "col": (len(kinds_gps) * len(gps_ws)
                                + len(kinds_sort) * len(sort_ws))}
            drop_col = pe_state["col"] + n_pe_cols
            mv_tiles = {}

            # mv block ranges: a larger first block keeps the PE fed (and
            # p-state ramped) through the y-DMA phase; later blocks pace on
            # pool buffer frees.
            mv_blk0 = int(os.environ.get("KCRPS_MV_BLK0", "2"))
            mv_ranges = []
            _g = 0
            while _g < n_pe_groups:
                n_b = mv_blk0 if not mv_ranges else mv_blk
                mv_ranges.append((_g, min(_g + n_b, n_pe_groups)))
                _g += n_b
            blk_of_group = {}
            for bi_, (a_, b_) in enumerate(mv_ranges):
                for g_ in range(a_, b_):
                    blk_of_group[g_] = bi_

            def emit_mv_dma(blk):
                g0, g1 = mv_ranges[blk]
                pool_ = mv0_pool if blk == 0 else mv_pool
                mt = pool_.tile([E + 1, (g1 - g0) * PE_GROUP], f16,
                                tag="mv0" if blk == 0 else "mv")
                eng = {"sync": nc.sync, "scalar": nc.scalar}[
                    os.environ.get("KCRPS_MV_ENG", "scalar")]
                eng.dma_start(
                    out=mt[:],
                    in_=mv.ap()[:, g0 * PE_GROUP:g1 * PE_GROUP])
                mv_tiles[blk] = (mt, g0)

            # PE consumption units: the first 2048-col group is split into
            # g0_split units with SEPARATE PSUM tiles so its first
            # activation starts after only a fraction of the matmuls
            # (PSUM dependencies are tile-granular).
            pe_units = []
            if pe_w:
                sub = PE_GROUP // g0_split
                for s_ in range(g0_split):
                    pe_units.append((s_ * sub, sub))
                for g_ in range(1, n_pe_groups):
                    pe_units.append((g_ * PE_GROUP, PE_GROUP))
            assert len(pe_units) == n_pe_cols

            def emit_pe_groups(n):
                for _ in range(n):
                    u = pe_state["next"]
                    if u >= len(pe_units):
                        return
                    pe_state["next"] += 1
                    coff, csize = pe_units[u]
                    blk = blk_of_group[coff // PE_GROUP]
                    assert blk in mv_tiles
                    mt, g0 = mv_tiles[blk]
                    off = coff - g0 * PE_GROUP
                    pt = ps_pool.tile([PART, csize], f32, tag="ps")
                    if u == 0:
                        # warmup matmuls on the tiny weights tile, into the
                        # first unit's PSUM (overwritten by the real
                        # start=True matmuls): ramps the PE p-state during
                        # the pipeline fill without claiming a PSUM buffer.
                        for _ in range(int(os.environ.get(
                                "KCRPS_PE_WARM", "0"))):
                            nc.tensor.matmul(out=pt[:, 0:PART], lhsT=wt[:],
                                             rhs=wt[:], start=True,
                                             stop=True)
                    for q in range(csize // 512):
                        nc.tensor.matmul(
                            out=pt[:, q * 512:(q + 1) * 512],
                            lhsT=wt[:],
                            rhs=mt[:, off + q * 512:off + (q + 1) * 512],
                            start=True, stop=True)
                    nc.scalar.activation(
                        out=pt[:], in_=pt[:],
                        func=mybir.ActivationFunctionType.Abs,
                        accum_out=acc[:, pe_state["col"]:
                                      pe_state["col"] + 1])
                    pe_state["col"] += 1

            # ---- DMA schedule: interleave gps and sort chunks so both
            # GPSIMD and the DVE sort start early.
            gps_off = []
            off = 0
            for w in gps_ws:
                gps_off.append(off)
                off += w
            sort_off = []
            for w in sort_ws:
                sort_off.append(off)
                off += w
            gps_tiles = [None] * len(gps_ws)
            sort_tiles = [None] * len(sort_ws)
            # build the DMA op list: sort chunks are split into two
            # 8-plane halves so the first sort layer can start after h1.
            dma_ops = []            # (kind, i, half)
            omode = os.environ.get("KCRPS_DMA_ORDER", "gs")
            gs = [("g", i, None) for i in range(len(gps_ws))]
            ss = []
            for i in range(len(sort_ws)):
                ss += [("s", i, 0), ("s", i, 1)]
            if omode == "sg":       # all sort halves, then gps
                dma_ops = ss + gs
            elif omode == "sA":     # h1, g0, h2, g1, ...
                dma_ops = []
                pool_ = ss + gs
                a, b = ss, gs
                while a or b:
                    if a:
                        dma_ops.append(a.pop(0))
                    if b:
                        dma_ops.append(b.pop(0))
            else:                   # "gs": g0, h1, h2, g1, ...
                a, b = gs, ss
                while a or b:
                    if a:
                        dma_ops.append(a.pop(0))
                    if b:
                        dma_ops.append(b.pop(0))
                    if b:
                        dma_ops.append(b.pop(0))
            wt_pos = int(os.environ.get("KCRPS_WT_POS", "0"))
            if pe_w and wt_pos == 0:
                nc.sync.dma_start(out=wt[:], in_=wd.ap())
            th_early = os.environ.get("KCRPS_TH_EARLY", "1") == "1"
            th_pos = int(os.environ.get("KCRPS_TH_POS", "1"))
            if th_early:
                dma_ops.insert(th_pos, ("t", 0, None))
            if pe_w:
                # early mv blocks feed the first PE groups; the rest are
                # issued after the y DMAs (all on the sync queue, so a
                # waiting mv DMA never blocks compute issue on ACT/DVE).
                mv_pre = [int(x) for x in os.environ.get(
                    "KCRPS_MV_PRE", "0,2,4").split(",") if x.strip()]
                for bi_, pos in enumerate(mv_pre):
                    dma_ops.insert(min(pos, len(dma_ops)), ("m", bi_, None))
            for oi, (kind, i, half) in enumerate(dma_ops):
                if pe_w and wt_pos == oi + 1:
                    nc.sync.dma_start(out=wt[:], in_=wd.ap())
                if kind == "t":
                    nc.sync.dma_start(out=th[:], in_=t.ap())
                    continue
                if kind == "m":
                    emit_mv_dma(i)
                    continue
                if kind == "g":
                    w, o = gps_ws[i], gps_off[i]
                    if gps_tiles[i] is None:
                        yt = y_pool.tile([PART, E * w], f16, tag="ygps")
                        gps_tiles[i] = (yt, w, o)
                    yt = gps_tiles[i][0]
                    nc.sync.dma_start(
                        out=yt[:], in_=y.ap()[:, o * E:(o + w) * E])
                else:
                    w, o = sort_ws[i], sort_off[i]
                    if sort_tiles[i] is None:
                        yt = ys_pool.tile([PART, E * w], f16, tag="ysort")
                        sort_tiles[i] = (yt, w, o)
                    yt = sort_tiles[i][0]
                    h = E // 2 * w
                    if half == 0:
                        nc.sync.dma_start(
                            out=yt[:, 0:h], in_=y.ap()[:, o * E:o * E + h])
                    else:
                        nc.sync.dma_start(
                            out=yt[:, h:2 * h],
                            in_=y.ap()[:, o * E + h:(o + w) * E])
            if not th_early:
                nc.sync.dma_start(out=th[:], in_=t.ap())
            ydt = None
            if pe_w:
                ydt = ys_pool.tile([PART, n_drop * pe_w], f16, tag="ydrop")
                nc.sync.dma_start(
                    out=ydt[:], in_=y.ap()[:, sbuf_cols * E:y_cols])
                # remaining mv blocks, paced by mv_pool buffer frees; they
                # only ever block the final acc DMA behind them.
                for blk in range(len(mv_ranges)):
                    if blk not in mv_tiles:
                        emit_mv_dma(blk)

            # ---- GPSIMD: all pair diffs of gps chunks ----------------------
            pb_tiles = []
            for yt, w, off0 in gps_tiles:
                pbt = pb_pool.tile([PART, 120 * w], f16, tag="pb")
                cur = 0
                for d in range(1, E):
                    n = E - d
                    nc.gpsimd.tensor_tensor(
                        pbt[:, cur * w:(cur + n) * w],
                        yt[:, 0:n * w],
                        yt[:, d * w:(d + n) * w],
                        AluOpType.subtract)
                    cur += n
                pb_tiles.append((pbt, yt, w, off0))

            # ---- DVE program -----------------------------------------------
            # interleave: sort layers (bulk), gps relu/mae groups (as GPSIMD
            # output becomes ready), drop rows, PE groups stream on ACT.
            def grid_view(tile_ap, grid, w):
                if grid == "16":
                    return tile_ap.rearrange("p (e f) -> p e f", f=w)
                a = {"2x8": 2, "4x4": 4, "8x2": 8}[grid]
                return tile_ap.rearrange("p (a b f) -> p a b f", a=a, f=w)

            def emit_sort(yt, w):
                """Batcher sort of the 16 e-planes of yt using the
                scattered-location plan (no passthrough copies): compared
                planes write to the opposite buffer, untouched planes stay
                put.  Returns (ta, tb); sorted plane k lives in
                bufs[_B_FINAL[k]] slot k."""
                ta = st_pool.tile([PART, E * w], f16, tag="sa")
                tb = st_pool.tile([PART, E * w], f16, tag="sb")
                bufs = (ta, tb)
                yv = grid_view(yt[:], "16", w)
                av = grid_view(ta[:], "16", w)
                # layer 0 reads yt in two plane halves (split DMA), all
                # outputs to A
                for lo, hi in ((0, 8), (8, 16)):
                    i_h = yv[:, lo:hi:2, :]
                    j_h = yv[:, lo + 1:hi:2, :]
                    nc.vector.tensor_tensor(
                        av[:, lo:hi:2, :], i_h, j_h, AluOpType.min)
                    nc.vector.tensor_tensor(
                        av[:, lo + 1:hi:2, :], i_h, j_h, AluOpType.max)

                def vw(bufi, expr):
                    g, osl, isl = expr
                    gv = grid_view(bufs[bufi][:], g, w)
                    if g == "16":
                        return gv[:, osl, :]
                    return gv[:, osl, isl, :]

                half = os.environ.get("KCRPS_SORT_INPLACE", "0") == "1"
                for ops in _SCATTER_OPS:
                    for bi, bj, ei, ej, pl, jpl in ops:
                        i_in = vw(bi, ei)
                        j_in = vw(bj, ej)
                        # min first (reads originals, writes the other
                        # buffer); max second, in place over in1
                        nc.vector.tensor_tensor(
                            vw(1 - bi, ei), i_in, j_in, AluOpType.min)
                        nc.vector.tensor_tensor(
                            vw(bj if half else 1 - bj, ej), i_in, j_in,
                            AluOpType.max)
                return bufs

            def emit_drop():
                dt_ = dr_pool.tile([PART, len(PE_DROP) * pe_w], f16,
                                   tag="dr")
                emit = [
                    (2, 0, 3),   # d=12: planes idx 0..1 vs 3..4
                    (3, 0, 4),   # d=13: idx 0..2 vs 4..6
                    (2, 0, 5),   # d=14: idx 0..1 vs 5..6
                    (1, 0, 6),   # d=15: idx 0 vs 6
                ]
                cur = 0
                for r, i0, i1 in emit:
                    nc.vector.tensor_tensor(
                        dt_[:, cur * pe_w:(cur + r) * pe_w],
                        ydt[:, i0 * pe_w:(i0 + r) * pe_w],
                        ydt[:, i1 * pe_w:(i1 + r) * pe_w],
                        AluOpType.max)
                    cur += r
                nc.vector.tensor_scalar(
                    out=dt_[:], in0=dt_[:], scalar1=0.0, scalar2=0.0,
                    op0=AluOpType.bypass, op1=AluOpType.add,
                    accum_out=acc[:, drop_col:drop_col + 1])

            # --- interleaved emission --------------------------------------
            # Column order must match host decode: per gps chunk
            # [relu x relu_g, gmae], then per sort chunk [coef x16, smae];
            # emission order differs, so allocate columns up-front.
            col_map = {}
            c = 0
            for gi in range(len(gps_ws)):
                for g in range(relu_g):
                    col_map[("gpsrelu", gi, g)] = c
                    c += 1
                col_map[("gmae", gi)] = c
                c += 1
            for si in range(len(sort_ws)):
                for k in range(E):
                    col_map[("coef", si, k)] = c
                    c += 1
                col_map[("smae", si)] = c
                c += 1
            assert c == pe_state["col"]

            def gps_consume(gi, g, scratch=None):
                # The elementwise relu output is unused (only accum_out
                # matters).  Writing it into the sort scratch region gives
                # the op a WAR hazard against the final sort layers, which
                # pins it late in the DVE stream -- the tile scheduler's
                # internal cost model underestimates GPSIMD time by ~2.4x
                # and otherwise hoists these between early sort layers,
                # head-of-line blocking the DVE for many microseconds.
                pbt, yt, w, off0 = pb_tiles[gi]
                g0, g1 = pb_groups[g]
                cc = col_map[("gpsrelu", gi, g)]
                n = (g1 - g0) * w
                out_ap = (scratch[:, 0:n] if scratch is not None
                          else pbt[:, g0 * w:g1 * w])
                nc.vector.tensor_scalar(
                    out=out_ap, in0=pbt[:, g0 * w:g1 * w],
                    scalar1=0.0, scalar2=0.0,
                    op0=AluOpType.max, op1=AluOpType.add,
                    accum_out=acc[:, cc:cc + 1])

            def gps_mae(gi):
                pbt, yt, w, off0 = pb_tiles[gi]
                mt = sc_pool.tile([PART, E * w], f16, tag="gmae")
                yv = yt[:].rearrange("p (e f) -> p e f", e=E)
                tb = (th[:, pe_w + off0:pe_w + off0 + w]
                      .unsqueeze(1).broadcast_to([PART, E, w]))
                mv_ = mt[:].rearrange("p (e f) -> p e f", e=E)
                nc.vector.tensor_tensor(mv_[:, :, :], yv[:, :, :], tb,
                                        AluOpType.max)
                cc = col_map[("gmae", gi)]
                nc.vector.tensor_scalar(
                    out=mt[:], in0=mt[:], scalar1=0.0, scalar2=0.0,
                    op0=AluOpType.bypass, op1=AluOpType.add,
                    accum_out=acc[:, cc:cc + 1])

            def sort_coef(si, bufs, w):
                for k in range(E):
                    sv = bufs[_B_FINAL[k]][:].rearrange(
                        "p (e f) -> p e f", e=E)
                    cc = col_map[("coef", si, k)]
                    nc.vector.tensor_scalar(
                        out=sv[:, k, :], in0=sv[:, k, :],
                        scalar1=float(2 * k - (E - 1)), scalar2=0.0,
                        op0=AluOpType.mult, op1=AluOpType.add,
                        accum_out=acc[:, cc:cc + 1])

            def sort_mae(si, yt, w, off0):
                # mae is permutation-invariant: read the original
                # (unsorted) planes straight from the DMA tile, in place
                # (yt is dead after layer 0 + this).
                yv = yt[:].rearrange("p (e f) -> p e f", e=E)
                tb = (th[:, pe_w + off0:pe_w + off0 + w]
                      .unsqueeze(1).broadcast_to([PART, E, w]))
                nc.vector.tensor_tensor(yv[:, :, :], yv[:, :, :], tb,
                                        AluOpType.max)
                cc = col_map[("smae", si)]
                nc.vector.tensor_scalar(
                    out=yt[:], in0=yt[:], scalar1=0.0,
                    scalar2=0.0,
                    op0=AluOpType.bypass, op1=AluOpType.add,
                    accum_out=acc[:, cc:cc + 1])

            # emission: interleave DVE work so it rarely stalls on GPSIMD,
            # and spread PE-group emission so mv DMA keeps ahead of PE.
            ngps = len(gps_tiles)
            assert len(sort_tiles) >= 1
            # kick a first batch of PE groups so ACT starts early
            emit_pe_groups(int(os.environ.get("KCRPS_EARLY_PE", "4")))
            if os.environ.get("KCRPS_GMAE0_EARLY", "1") == "1":
                # gps chunk 0's mae needs only ygps0+th (land early): fills
                # the DVE idle window before the first sort DMA completes
                gps_mae(0)

            # sort chunks at high priority: the scheduler slots gps
            # consumers into DVE idle moments but prefers sort work the
            # moment its data lands.
            scratches = []
            for si, (yts, ws, offs) in enumerate(sort_tiles):
                with tc.high_priority():
                    bufs = emit_sort(yts, ws)
                    sort_mae(si, yts, ws, offs)
                    sort_coef(si, bufs, ws)
                scratches.append(bufs[0])
                emit_pe_groups(4)
            if pe_w:
                emit_drop()

            # gps consumers last; their dummy outputs write into the final
            # sort scratch to pin them after the sort (see gps_consume).
            pin = scratches[-1]
            for gi in range(ngps):
                for g in range(relu_g):
                    gps_consume(gi, g, scratch=pin)
                    emit_pe_groups(2)
                if gi > 0 or os.environ.get("KCRPS_GMAE0_EARLY", "1") != "1":
                    gps_mae(gi)

            emit_pe_groups(len(pe_units) - pe_state["next"])

            out_eng = {"sync": nc.sync, "scalar": nc.scalar,
                       "gpsimd": nc.gpsimd}[
                os.environ.get("KCRPS_OUT_ENG", "sync")]
            if os.environ.get("KCRPS_OUT_SPLIT", "0") == "1" and pe_w:
                # two-piece result DMA: everything but the last-written
                # ACT column ships while the final PSUM group is still
                # being consumed
                cut = drop_col - 1
                out_eng.dma_start(out=out.ap()[:, 0:cut], in_=acc[:, 0:cut])
                out_eng.dma_start(out=out.ap()[:, cut:ncol],
                                  in_=acc[:, cut:ncol])
            else:
                out_eng.dma_start(out=out.ap(), in_=acc[:])
    nc.compile()
    nc._kcrps_meta = (pe_w, tuple(gps_ws), tuple(sort_ws), relu_g, ncol)
    return nc


def kernel(y_pred, y_target, weights, scale):
    global LAST_EXEC_NS, LAST_NC
    from concourse.bass_utils import run_bass_kernel_spmd

    pe_w = _pe_w()
    gps_ws = _gps_ws()
    sort_ws = _sort_ws()
    relu_g = int(os.environ.get("KCRPS_RELU_GROUPS", "3"))
    key = ("v2", pe_w, tuple(gps_ws), tuple(sort_ws), relu_g, PE_GROUP)
    if key not in _CACHE:
        _CACHE[key] = _build_nc(pe_w, gps_ws, sort_ws)
    nc = _CACHE[key]
    LAST_NC = nc

    y_pred = np.asarray(y_pred, dtype=np.float32)
    y_target = np.asarray(y_target, dtype=np.float32)
    weights = np.asarray(weights, dtype=np.float32)
    scale = np.asarray(scale, dtype=np.float32)

    ghat = (scale[None, :, None] * weights[None, None, :])     # (1, V, P)
    yh = (y_pred * ghat[..., None]).astype(np.float16)         # (B, V, P, E)
    th = (y_target * ghat).astype(np.float16)                  # (B, V, P)

    n_drop = len(PE_DROP_PLANES)
    sbuf_cols = sum(gps_ws) + sum(sort_ws)
    gps_tot = sum(gps_ws)

    if pe_w:
        W = np.zeros((E + 1, PART), np.float16)
        for m, (d, i) in enumerate(PE_PAIRS):
            W[i, m] = 1.0
            W[i + d, m] = -1.0
        for k in range(E):
            W[E, 112 + k] = 1.0
            W[k, 112 + k] = -1.0

    in_maps = []
    C_gps = np.zeros(E, np.float64)
    C_sbuf = np.zeros(E, np.float64)
    C_pe = np.zeros(E, np.float64)
    T1_sbuf = 0.0
    for c in range(NCORES):
        sl = slice(c * PC, (c + 1) * PC)
        arr = yh[:, :, sl, :].reshape(PART, FREE, E)
        tharr = th[:, :, sl].reshape(PART, FREE)
        segs = []
        off = pe_w
        for w in list(gps_ws) + list(sort_ws):
            seg = arr[:, off:off + w, :].transpose(0, 2, 1)    # (PART, E, w)
            segs.append(seg.reshape(PART, E * w))
            off += w
        imap = {}
        if pe_w:
            dseg = (arr[:, 0:pe_w, :][:, :, PE_DROP_PLANES]
                    .transpose(0, 2, 1).reshape(PART, n_drop * pe_w))
            segs.append(dseg)
            mvy = arr[:, 0:pe_w, :].reshape(PART * pe_w, E).T  # (E, S)
            mvt = tharr[:, 0:pe_w].reshape(1, PART * pe_w)
            imap["mv"] = np.ascontiguousarray(
                np.concatenate([mvy, mvt], axis=0).astype(np.float16))
            imap["wm"] = W
            C_pe += arr[:, 0:pe_w, :].astype(np.float64).sum(axis=(0, 1))
        imap["y"] = np.ascontiguousarray(np.concatenate(segs, axis=1))
        imap["t"] = np.ascontiguousarray(tharr)
        in_maps.append(imap)
        C_gps += (arr[:, pe_w:pe_w + gps_tot, :]
                  .astype(np.float64).sum(axis=(0, 1)))
        C_sbuf += arr[:, pe_w:, :].astype(np.float64).sum(axis=(0, 1))
        T1_sbuf += tharr[:, pe_w:].astype(np.float64).sum()

    res = run_bass_kernel_spmd(
        nc, in_maps, core_ids=list(range(NCORES)), trace=False)
    LAST_EXEC_NS = res.exec_time_ns

    n_pe_groups = PART * pe_w // PE_GROUP if pe_w else 0
    n_pe_cols = ((n_pe_groups - 1
                  + int(os.environ.get("KCRPS_G0_SPLIT", "1")))
                 if pe_w else 0)
    R_relu = M_gmae = M_smae = 0.0
    PAIR_sort = 0.0
    A_abs = A_mae = M_drop = 0.0
    for c in range(NCORES):
        a = res.results[c]["acc"].astype(np.float64)
        cc = 0
        for gi in range(len(gps_ws)):
            for g in range(relu_g):
                R_relu += a[:, cc].sum()
                cc += 1
            M_gmae += a[:, cc].sum()
            cc += 1
        for si in range(len(sort_ws)):
            for k in range(E):
                PAIR_sort += a[:, cc].sum()
                cc += 1
            M_smae += a[:, cc].sum()
            cc += 1
        if pe_w:
            pe_cols = a[:, cc:cc + n_pe_cols]
            A_abs += pe_cols[0:112, :].sum()       # matrix pair rows
            A_mae += pe_cols[112:128, :].sum()     # matrix mae rows
            M_drop += a[:, cc + n_pe_cols].sum()  # dropped pair rows

    # linear corrections (exact, fp64, from fp16 inputs)
    L_gps = 0.0          # sum over all (d,i) pairs of (C_i - C_{i+d})
    for d in range(1, E):
        for i in range(E - d):
            L_gps += C_gps[i] - C_gps[i + d]
    L_drop = 0.0
    for d, i in PE_DROP:
        L_drop += C_pe[i] + C_pe[i + d]

    PAIR_total = (A_abs + PAIR_sort
                  + 2.0 * R_relu - L_gps
                  + 2.0 * M_drop - L_drop)
    MAE_total = (A_mae + 2.0 * (M_gmae + M_smae)
                 - E * T1_sbuf - C_sbuf.sum())
    npoints = weights.astype(np.float64).sum()
    result = (MAE_total / E - PAIR_total / (E * E)) / (npoints * B)
    return np.float32(result)
